# revision 1
# baseline (speedup 1.0000x reference)
"""BitLinear TRN2 kernel v5: y = x @ W(pweight,nweight)^T + bias.

Sharding: 2 token-shards x 4 out-feature shards (column-parallel linear,
no collectives). Each core: 8192 tokens x 512 out features.

Per core: x pre-transposed [I, TQ=8192] bf16, streamed in 4 slabs of 2048
tokens (one [128, 16, 2048] tile, 4 chunk-DMAs, double-buffered, so matmuls
carry early-resolving semaphore waits instead of just-in-time ones).
Weights: host-precast bf16 n-planes for this core's 512 o-rows, loaded on
the gpsimd ring (pw) and scalar ring (nw) so prep never queues behind x;
prepped o-tile by o-tile during slab 0 (sigmoid on ACT, subtract +
c_n-combine on DVE) into four resident [128, 16, 128] wT tiles (split per
o-tile so the first ldweights does not wait on the whole prep). Main loop
is weight-stationary: per (slab, o-tile), 16 x (ldweights + 4 matmuls into
4 PSUM banks of [128 o, 512 t]); 2 bank-parities overlap accumulate/drain.
Output written as yT [512, 8192] bf16 (bias added during the DVE PSUM
drain), upcast + transposed on host.
"""

import numpy as np

import concourse.bass as bass
import concourse.mybir as mybir
import concourse.tile as tile
from concourse import bacc
from concourse.bass_utils import run_bass_kernel_spmd

N_CORES = 8
T, I, O, NB = 16384, 2048, 2048, 4
R, C = 2, 4  # token shards x out-feature shards
TQ = T // R  # 8192 tokens per core
OC = O // C  # 512 out features per core
P = 128
N_IT = I // P  # 16 i-tiles
N_OT = OC // P  # 4 o-tiles per core
N_H = 2  # i-halves per prep DMA
HIT = N_IT // N_H  # 8 i-tiles per half
TSLAB = 2048  # tokens per slab
N_SLAB = TQ // TSLAB  # 4 slabs
TCH = 512  # moving free size per matmul
N_TC = TSLAB // TCH  # 4 t-chunks = 4 PSUM banks per o-tile
DT = mybir.dt.bfloat16
F32 = mybir.dt.float32

LDW_MODE = 1

_BUILT = None


def _build_bass(reps=1, mode="full"):
    nc = bacc.Bacc("TRN2", debug=False, num_devices=N_CORES)

    xt_d = nc.dram_tensor("xt", [I, TQ], DT, kind="ExternalInput").ap()
    # [N_OT, N_H, P(part=i within half), HIT, NB, P(o)]: per-(ot,h) DMA is one
    # 8KB-contiguous descriptor per partition
    pw_d = nc.dram_tensor(
        "pw", [N_OT, N_H, P, HIT, NB, P], DT, kind="ExternalInput"
    ).ap()
    nw_d = nc.dram_tensor(
        "nw", [N_OT, N_H, P, HIT, NB, P], DT, kind="ExternalInput"
    ).ap()
    cv_d = nc.dram_tensor("cvec", [P, NB], DT, kind="ExternalInput").ap()
    bias_d = nc.dram_tensor("bias", [P, N_OT], F32, kind="ExternalInput").ap()
    y_d = nc.dram_tensor("y", [OC, TQ], DT, kind="ExternalOutput").ap()

    with tile.TileContext(nc) as tc:
        with (
            tc.tile_pool(name="const", bufs=1) as const_pool,
            tc.tile_pool(name="xs", bufs=2) as xs_pool,
            tc.tile_pool(name="wio", bufs=2) as wio_pool,
            tc.tile_pool(name="sig", bufs=1) as sig_pool,
            tc.tile_pool(name="acc", bufs=1) as acc_pool,
            tc.tile_pool(name="tmp", bufs=1) as tmp_pool,
            tc.tile_pool(name="wT", bufs=1) as wt_pool,
            tc.tile_pool(name="yo", bufs=2) as yo_pool,
            tc.tile_pool(name="mm_ps", bufs=1, space="PSUM") as mm_ps,
        ):
            cv_sb = const_pool.tile([P, NB], DT)
            nc.sync.dma_start(cv_sb[:], cv_d[:])
            bias_sb = const_pool.tile([P, N_OT], F32)
            nc.sync.dma_start(bias_sb[:], bias_d[:])

            for _rep in range(reps):
                wTs = [
                    wt_pool.tile([P, N_IT, P], DT, tag=f"wT{ot}", name=f"wT{ot}")
                    for ot in range(N_OT)
                ]

                # ---------- weight prep (during slab 0) ----------
                if mode != "mm":
                    for ot in range(N_OT):
                        for h in range(N_H):
                            pwn = wio_pool.tile([P, HIT, NB, P], DT, tag="pwn")
                            nc.gpsimd.dma_start(pwn[:], pw_d[ot, h])
                            nwn = wio_pool.tile([P, HIT, NB, P], DT, tag="nwn")
                            nc.scalar.dma_start(nwn[:], nw_d[ot, h])
                            sp = sig_pool.tile([P, HIT, NB, P], DT, tag="sp")
                            nc.scalar.activation(
                                sp[:], pwn[:], mybir.ActivationFunctionType.Sigmoid
                            )
                            sn = sig_pool.tile([P, HIT, NB, P], DT, tag="sn")
                            nc.scalar.activation(
                                sn[:], nwn[:], mybir.ActivationFunctionType.Sigmoid
                            )
                            nc.vector.tensor_sub(out=sp[:], in0=sp[:], in1=sn[:])
                            soft = sp
                            acc = acc_pool.tile([P, HIT, P], F32, tag="acc")
                            for n in range(NB):
                                cb = cv_sb[:, n : n + 1, None].to_broadcast(
                                    (P, HIT, P)
                                )
                                if n == 0:
                                    nc.vector.tensor_tensor(
                                        acc[:],
                                        soft[:, :, n, :],
                                        cb,
                                        mybir.AluOpType.mult,
                                    )
                                else:
                                    tmp = tmp_pool.tile([P, HIT, P], DT, tag="tmp")
                                    nc.vector.tensor_tensor(
                                        tmp[:],
                                        soft[:, :, n, :],
                                        cb,
                                        mybir.AluOpType.mult,
                                    )
                                    dst = (
                                        wTs[ot][:, h * HIT : (h + 1) * HIT, :]
                                        if n == NB - 1
                                        else acc[:]
                                    )
                                    nc.vector.tensor_tensor(
                                        dst, acc[:], tmp[:], mybir.AluOpType.add
                                    )

                if mode == "w":
                    continue
                # ---------- main ----------
                for sl in range(N_SLAB):
                    tcols = slice(sl * TSLAB, (sl + 1) * TSLAB)
                    xslab = xs_pool.tile(
                        [P, N_IT, TSLAB], DT, tag="xslab", name="xslab"
                    )
                    for c in range(N_TC):
                        tc0 = sl * TSLAB + c * TCH
                        nc.sync.dma_start(
                            xslab[:, :, c * TCH : (c + 1) * TCH],
                            xt_d[:, tc0 : tc0 + TCH].rearrange(
                                "(it p) t -> p it t", p=P
                            ),
                        )
                    for ot in range(N_OT):
                        par = (sl * N_OT + ot) % 2
                        banks = [
                            mm_ps.tile(
                                [P, TCH], F32, tag=f"ps{par}{c}", name=f"ps{par}{c}"
                            )
                            for c in range(N_TC)
                        ]
                        for it in range(N_IT):
                            lw = wTs[ot][:, it, :]
                            if LDW_MODE:
                                nc.tensor.ldweights(lw)
                            for c in range(N_TC):
                                mm = nc.tensor.matmul(
                                    banks[c][:],
                                    lw,
                                    xslab[:, it, c * TCH : (c + 1) * TCH],
                                    start=(it == 0),
                                    stop=(it == N_IT - 1),
                                )
                                if LDW_MODE:
                                    mm.ldweights = False
                        yt = yo_pool.tile([P, TSLAB], DT, tag="yt")
                        bb = bias_sb[:, ot : ot + 1].to_broadcast((P, TCH))
                        for c in range(N_TC):
                            nc.vector.tensor_tensor(
                                yt[:, c * TCH : (c + 1) * TCH],
                                banks[c][:],
                                bb,
                                mybir.AluOpType.add,
                            )
                        nc.sync.dma_start(y_d[ot * P : (ot + 1) * P, tcols], yt[:])

    nc.compile()
    return nc


def get_built():
    global _BUILT
    if _BUILT is None:
        _BUILT = _build_bass()
    return _BUILT


def make_in_maps(
    input, pweight, nweight, exps, bexps, mask_weight, scale, pbias, nbias, biasscale
):
    import ml_dtypes

    input = np.asarray(input, dtype=np.float32)
    pweight = np.asarray(pweight, dtype=np.float32)
    nweight = np.asarray(nweight, dtype=np.float32)
    exps = np.asarray(exps, dtype=np.float32)
    bexps = np.asarray(bexps, dtype=np.float32)
    mask_weight = np.asarray(mask_weight, dtype=np.float32)
    scale = np.asarray(scale, dtype=np.float32)
    pbias = np.asarray(pbias, dtype=np.float32)
    nbias = np.asarray(nbias, dtype=np.float32)
    biasscale = np.asarray(biasscale, dtype=np.float32)

    mask = 1.0 / (1.0 + np.exp(-mask_weight))
    c4 = (exps * mask * scale[0]).astype(np.float32)
    cvec = np.ascontiguousarray(np.broadcast_to(c4, (P, NB)).astype(ml_dtypes.bfloat16))

    bias_raw = (pbias - nbias) @ bexps  # [O]
    step = float(2**NB - 1)
    b = np.clip(bias_raw, -1.0, 1.0)
    bias = (np.round(np.abs(b) * step) / step * np.sign(b)) * biasscale[0]

    def wlayout(w):
        # per-core [OC=512, I, NB] -> [N_OT, N_H, P(part), HIT, NB, P(o)]
        a = w.reshape(N_OT, P, N_H, HIT, P, NB)  # [ot, o, h, hit, p, n]
        a = a.transpose(0, 2, 4, 3, 5, 1)  # [ot, h, p, hit, n, o]
        return np.ascontiguousarray(a.astype(ml_dtypes.bfloat16))

    x = input.reshape(T, I)
    xts = []
    for tr in range(R):
        sl = slice(tr * TQ, (tr + 1) * TQ)
        xts.append(np.ascontiguousarray(x[sl].T.astype(ml_dtypes.bfloat16)))

    in_maps = []
    for core in range(N_CORES):
        tr, oc = divmod(core, C)
        osl = slice(oc * OC, (oc + 1) * OC)
        in_maps.append(
            {
                "xt": xts[tr],
                "pw": wlayout(pweight[osl]),
                "nw": wlayout(nweight[osl]),
                "cvec": cvec,
                "bias": np.ascontiguousarray(
                    bias[osl].reshape(N_OT, P).T.astype(np.float32)
                ),
            }
        )
    return in_maps


def gather_output(results):
    y = np.empty((T, O), dtype=np.float32)
    for core, r in enumerate(results):
        tr, oc = divmod(core, C)
        y[tr * TQ : (tr + 1) * TQ, oc * OC : (oc + 1) * OC] = (
            r["y"].astype(np.float32).T
        )
    return y.reshape(8, T // 8, O)


def kernel(**inputs) -> np.ndarray:
    in_maps = make_in_maps(**inputs)
    nc = get_built()
    res = run_bass_kernel_spmd(nc, in_maps, core_ids=list(range(N_CORES)))
    return gather_output(res.results)

